# revision 1
# baseline (speedup 1.0000x reference)
"""Trainium2 Bass kernel for DifferentiableExtrusion.

voxels[b,d,h,w] = depth_mask[b,d] * max_n(valid_n * sigmoid(-100*sdf_n(h,w)))
B=4, N=32 polygons (P=16 vertices), V=128 grid, D=128.

Sharding: 8 cores = (b, half). Each core handles half of batch b's *valid*
polygons, computes its partial combined mask, extrudes along D and writes a
partial [D,H,W] voxel block. Host max-reduces the two halves of each b.

Per-edge distance (linear-form coefficients precomputed on host):
  h  = -((p-v0).e)/k + k/2,  k = sqrt(|e|^2 + eps)
  g  = relu(|h| - k/2)       (= k*|t* - clamp(t*,0,1)|)
  w  = (p-v0).perp(e)/k
  d2 = g^2 + w^2             (error vs reference ~1e-7)

Inside test: even-odd crossings. The host quantizes each edge's row
intersection to a column threshold (exact f32 comparisons, identical to the
reference's), builds a per-row histogram with parity-preserving row-start
corrections, and the device turns it into per-pixel crossing parity with a
single prefix-scan along the free dim.

mask = sigmoid(100 * d * (2*inside - 1)); combined = max over polys.

Dtypes: linear forms and everything feeding a cancellation are f32; after the
relu the per-edge chain (g, g^2, w^2, d2, min) is bf16 - all rounding there
is relative, so the final sigmoid error stays ~1e-3 absolute.

Extrusion: staged[h, (d,w)] = comb[h,w] * depth[d] is built on the DVE and
gpsimd engines in eight alternating chunks, each written out by a
partition-aligned DMA (contiguous bytes per partition - the ideal pattern).
The polygon-max tree runs its rounds in free-dim halves so each half's
cross-partition copy overlaps the other half's max. The device output layout
is [H, D*W]; the host transposes d<->h while max-combining core pairs.
"""

import numpy as np

import concourse.bacc as bacc
import concourse.tile as tile
from concourse import mybir
from concourse.bass_utils import run_bass_kernel_spmd
from concourse.tile_rust import add_dep_helper

V = 128
P = 16
SHARP = 100.0
EPS = 1e-8
NCORES = 8

F32 = mybir.dt.float32
BF16 = mybir.dt.bfloat16
I32 = mybir.dt.int32
AF = mybir.ActivationFunctionType
OP = mybir.AluOpType

# ----------------------------------------------------------------------------
# Host-side: polygon -> per-edge coefficients + crossing thresholds
# ----------------------------------------------------------------------------

def _poly_coeffs(poly):
    vmask = poly.sum(axis=1) != 0.0
    K = int(vmask.sum())
    order = np.argsort((~vmask).astype(np.int32), kind="stable")
    pv = poly[order].astype(np.float64)
    idx = np.arange(P)
    nxt = np.where(idx == K - 1, 0, idx + 1)
    v0 = pv
    v1 = pv[nxt]
    valid_e = idx < K if K >= 3 else np.zeros(P, bool)

    ex = v1[:, 0] - v0[:, 0]
    ey = v1[:, 1] - v0[:, 1]
    s2 = ex * ex + ey * ey + EPS
    k = np.sqrt(s2)

    hx = -ex / k
    hy = -ey / k
    hc = (v0[:, 0] * ex + v0[:, 1] * ey) / k + k / 2.0
    wx = -ey / k
    wy = ex / k
    wc = (ey * v0[:, 0] - ex * v0[:, 1]) / k

    hx = np.where(valid_e, hx, 0.0)
    hy = np.where(valid_e, hy, 0.0)
    hc = np.where(valid_e, hc, 1e3)
    wx = np.where(valid_e, wx, 0.0)
    wy = np.where(valid_e, wy, 0.0)
    wc = np.where(valid_e, wc, 0.0)
    khalf = np.where(valid_e, k / 2.0, 0.0)

    # crossing thresholds, float32 ops mirroring the reference bit-for-bit:
    # thr[e, y] = #{grid columns j with inter_x > x_j}; 0 when !y_crosses
    x32 = np.arange(V, dtype=np.float32) / np.float32(V - 1)
    y32 = x32
    x0 = v0[:, 0].astype(np.float32)[:, None]
    y0 = v0[:, 1].astype(np.float32)[:, None]
    x1 = v1[:, 0].astype(np.float32)[:, None]
    y1 = v1[:, 1].astype(np.float32)[:, None]
    yrow = y32[None, :]
    yc = ((y0 <= yrow) & (y1 > yrow)) | ((y1 <= yrow) & (y0 > yrow))
    t = (yrow - y0) / (y1 - y0 + np.float32(EPS))
    ix = x0 + (x1 - x0) * t                                   # (P, V) f32
    yc = yc & valid_e[:, None]
    thr = (ix[:, :, None] > x32[None, None, :]).sum(axis=2)   # (P, V) ints
    thr = np.where(yc, thr, 0)

    return dict(hx=hx, hy=hy, hc=hc, wx=wx, wy=wy, wc=wc, khalf=khalf,
                thr=thr)


def _crossing_hist(thrs):
    """thrs: (n_edges, V) thresholds for one polygon.
    Returns H: (V, V) float64 such that forward-cumsum of H.ravel() has, at
    position (y, j), the parity of #crossings for pixel (y, j)."""
    Vn = V
    H = np.zeros((Vn, Vn))
    carry = 0.0  # running total of all previous H entries (exact ints)
    for y in range(Vn):
        th = thrs[:, y]
        hist = np.bincount(th[(th >= 1) & (th <= Vn - 1)], minlength=Vn)
        cnt0 = int((th >= 1).sum())       # crossings at column 0
        H[y, 1:] = hist[1:]
        H[y, 0] = cnt0 + (carry % 2)      # parity-preserving row reset
        carry += H[y].sum()
    return H


# ----------------------------------------------------------------------------
# Blob layout: one packed [128, NTOT] f32 input
# ----------------------------------------------------------------------------

def _blob_offsets(YY):
    FD = YY * V
    o = {}
    o["xb"] = 0                      # (YY, V) x coordinates
    o["hx"] = FD
    o["wx"] = FD + P
    o["nkh"] = FD + 2 * P            # -k/2 per edge
    o["hyc"] = FD + 3 * P            # (P, YY) hy*y + hc
    o["wyc"] = o["hyc"] + P * YY
    o["hist"] = o["wyc"] + P * YY    # (YY, V) crossing histogram
    o["dv"] = o["hist"] + FD         # (V,) depth indicator, replicated rows
    o["total"] = o["dv"] + V
    return o


def _build_device(npoly):
    G = 128 // npoly
    YY = V // G
    FD = YY * V
    o = _blob_offsets(YY)

    nc = bacc.Bacc()
    blob = nc.declare_dram_parameter("blob", [128, o["total"]], F32,
                                     isOutput=False)
    vox = nc.declare_dram_parameter("vox", [V, V * V], F32, isOutput=True)

    with tile.TileContext(nc) as tc:
        with (
            tc.tile_pool(name="const", bufs=1) as cpool,
            tc.tile_pool(name="work", bufs=3) as wpool,
            tc.tile_pool(name="acc", bufs=1) as apool,
        ):
            s_blob = cpool.tile([128, o["total"]], F32, tag="blob")
            # split the load so the edge loop only waits for its sections
            nc.sync.dma_start(out=s_blob[:, : o["hist"]],
                              in_=blob[:, : o["hist"]])
            nc.sync.dma_start(out=s_blob[:, o["hist"] : o["dv"]],
                              in_=blob[:, o["hist"] : o["dv"]])
            nc.sync.dma_start(out=s_blob[:, o["dv"] :],
                              in_=blob[:, o["dv"] :])

            s_xb = s_blob[:, o["xb"] : o["xb"] + FD].rearrange(
                "p (yy x) -> p yy x", yy=YY)

            def pscal(name, e):
                return s_blob[:, o[name] + e : o[name] + e + 1]

            def bcast(name, e):
                sl = s_blob[:, o[name] + e * YY : o[name] + (e + 1) * YY]
                return sl.unsqueeze(2).broadcast_to([128, YY, V])

            macc = apool.tile([128, YY, V], BF16, tag="macc")
            nc.vector.memset(macc, 1e9)

            for e in range(P):
                hbuf = wpool.tile([128, YY, V], F32, tag="hbuf")
                gbuf = wpool.tile([128, YY, V], BF16, tag="gbuf")
                gsq = wpool.tile([128, YY, V], BF16, tag="gsq")
                wbuf = wpool.tile([128, YY, V], BF16, tag="wbuf")
                wsq = wpool.tile([128, YY, V], BF16, tag="wsq")

                # h = hx*x + (hy*y + hc)            [DVE f32]
                nc.vector.scalar_tensor_tensor(
                    hbuf, s_xb, pscal("hx", e), bcast("hyc", e),
                    OP.mult, OP.add)
                # |h|                                [ACT f32]
                nc.scalar.activation(hbuf, hbuf, AF.Abs)
                # g = relu(|h| - k/2) -> bf16        [ACT]
                nc.scalar.activation(gbuf, hbuf, AF.Relu,
                                     bias=pscal("nkh", e))
                # g^2                                [ACT bf16]
                nc.scalar.activation(gsq, gbuf, AF.Square)
                # w = wx*x + (wy*y + wc)             [DVE f32]
                nc.vector.scalar_tensor_tensor(
                    wbuf, s_xb, pscal("wx", e), bcast("wyc", e),
                    OP.mult, OP.add)
                # w^2 -> bf16: ACT takes a quarter, GPSIMD the rest
                Q = YY // 4
                nc.scalar.activation(wsq[:, :Q], wbuf[:, :Q], AF.Square)
                nc.gpsimd.tensor_tensor(
                    wsq[:, Q:], wbuf[:, Q:], wbuf[:, Q:], OP.mult)
                # d2 = g^2 + w^2: DVE 3/8, GPSIMD 5/8
                S = (YY * 3) // 8
                nc.vector.tensor_tensor(
                    gsq[:, :S], gsq[:, :S], wsq[:, :S], OP.add)
                nc.gpsimd.tensor_tensor(
                    gsq[:, S:], gsq[:, S:], wsq[:, S:], OP.add)
                # macc = min(macc, d2)               [DVE bf16 2x]
                nc.vector.tensor_tensor(macc, macc, gsq, OP.min)

            # crossing parity: S = cumsum(hist); inside = S mod 2
            s_hist = s_blob[:, o["hist"] : o["hist"] + FD]
            scan = apool.tile([128, FD], F32, tag="scan")
            nc.vector.tensor_tensor_scan(
                scan, s_hist, s_hist, 0.0, OP.add, OP.bypass)
            # sgn = 2*(S & 1) - 1  in {-1, +1} (parity via int cast; scan
            # values are small exact integers, and f32 mod isn't HW-valid)
            si = apool.tile([128, FD], I32, tag="si")
            nc.vector.tensor_copy(si, scan)
            nc.vector.tensor_scalar(si, si, 1, None, OP.bitwise_and)
            sgn = apool.tile([128, FD], F32, tag="sgn")
            nc.vector.tensor_copy(sgn, si)
            nc.vector.tensor_scalar(sgn, sgn, 2.0, -1.0, OP.mult, OP.add)

            # mask = sigmoid(100 * sqrt(macc) * sgn)
            rbuf = apool.tile([128, YY, V], F32, tag="rbuf")
            nc.scalar.activation(rbuf, macc, AF.Sqrt)
            nc.vector.tensor_tensor(
                rbuf, rbuf, sgn.rearrange("p (yy x) -> p yy x", yy=YY),
                OP.mult)
            nc.scalar.activation(rbuf, rbuf, AF.Sigmoid, scale=SHARP)

            # max over poly slots (tree over partition blocks)
            nparts = 128
            cur = rbuf
            HYY = YY // 2
            prev_insts = [None, None]
            while nparts > G:
                half = nparts // 2
                tmp = wpool.tile([half, YY, V], F32, tag="treetmp")
                for xh in range(2):
                    sl = slice(xh * HYY, (xh + 1) * HYY)
                    d_i = nc.sync.dma_start(
                        out=tmp[:half, sl], in_=cur[half:nparts, sl])
                    if prev_insts[xh] is not None:
                        add_dep_helper(d_i.ins, prev_insts[xh].ins,
                                       reason="tree round reads prior max")
                    t_i = nc.vector.tensor_tensor(
                        cur[:half, sl], cur[:half, sl], tmp[:half, sl],
                        OP.max)
                    add_dep_helper(t_i.ins, d_i.ins,
                                   reason="tree max reads dma")
                    prev_insts[xh] = t_i
                nparts = half
            prev_inst = prev_insts[1]
            comb_dep0 = prev_insts[0]

            # reshape [G, YY, V] -> [V, V] (partition = grid row)
            comb = apool.tile([128, V], F32, tag="comb")
            comb_dma = nc.sync.dma_start(out=comb, in_=cur[:G])
            add_dep_helper(comb_dma.ins, prev_inst.ins,
                           reason="reshape reads final tree max")
            add_dep_helper(comb_dma.ins, comb_dep0.ins,
                           reason="reshape reads final tree max half0")

            # extrusion: staged[h, (d, w)] = comb[h, w] * dv[d] built on
            # the (otherwise idle) gpsimd engine in halves, then one
            # partition-aligned DMA per half writes contiguous 64KB rows -
            # the ideal DMA pattern. Output layout is [H, D*W]; the host
            # transposes d<->h when combining core pairs.
            s_dv = s_blob[:, o["dv"] : o["dv"] + V]
            HD = V // 8
            for i in range(8):
                stg = wpool.tile([128, HD, V], F32, tag="stg")
                eng = nc.vector if i % 2 == 0 else nc.gpsimd
                m_i = eng.tensor_tensor(
                    stg,
                    comb.unsqueeze(1).broadcast_to([128, HD, V]),
                    s_dv[:, i * HD : (i + 1) * HD].unsqueeze(2).broadcast_to(
                        [128, HD, V]),
                    OP.mult)
                add_dep_helper(m_i.ins, comb_dma.ins,
                               reason="staging reads comb")
                nc.sync.dma_start(
                    out=vox[:, i * HD * V : (i + 1) * HD * V], in_=stg)

    nc.compile()
    return nc


_NC_CACHE = {}


def _get_nc(npoly):
    if npoly not in _NC_CACHE:
        _NC_CACHE[npoly] = _build_device(npoly)
    return _NC_CACHE[npoly]


# ----------------------------------------------------------------------------
# Host entry point
# ----------------------------------------------------------------------------

LAST_RESULTS = None


def kernel(polygons, attributes, validity_scores, _trace=False):
    global LAST_RESULTS
    polygons = np.asarray(polygons)
    attributes = np.asarray(attributes)
    validity_scores = np.asarray(validity_scores)
    B, N, _, _ = polygons.shape
    assert (B, N) == (4, 32)

    core_polys = []
    for b in range(B):
        valid = [n for n in range(N) if validity_scores[b, n] >= 0.5]
        h = (len(valid) + 1) // 2
        core_polys.append([(b, n) for n in valid[:h]])
        core_polys.append([(b, n) for n in valid[h:]])

    maxp = max(len(cp) for cp in core_polys)
    npoly = 4
    while npoly < maxp:
        npoly *= 2
    npoly = min(npoly, 16)
    assert maxp <= npoly, f"core poly count {maxp} exceeds {npoly}"

    G = 128 // npoly
    YY = V // G
    o = _blob_offsets(YY)
    nc = _get_nc(npoly)

    x32 = np.arange(V, dtype=np.float32) / np.float32(V - 1)
    y64 = np.arange(V, dtype=np.float64) / (V - 1)

    norm = np.clip(attributes[:, 0].astype(np.float32), 0.0, 1.0)
    hv = np.clip(np.rint(norm * np.float32(V)).astype(np.int32), 1, V)

    in_maps = []
    for c in range(NCORES):
        b = c // 2
        plist = core_polys[c]
        blob = np.zeros((128, o["total"]), np.float64)
        blob[:, o["xb"] : o["xb"] + YY * V] = np.tile(x32, YY)[None, :]

        for s, (pb, pn) in enumerate(plist):
            cf = _poly_coeffs(np.asarray(polygons[pb, pn], np.float32))
            H = _crossing_hist(cf["thr"])            # (V, V)
            rows = slice(s * G, (s + 1) * G)
            blob[rows, o["hx"] : o["hx"] + P] = cf["hx"][None, :]
            blob[rows, o["wx"] : o["wx"] + P] = cf["wx"][None, :]
            blob[rows, o["nkh"] : o["nkh"] + P] = -cf["khalf"][None, :]
            for g in range(G):
                p = s * G + g
                ys = y64[g * YY : (g + 1) * YY]
                blob[p, o["hyc"] : o["hyc"] + P * YY] = (
                    cf["hy"][:, None] * ys[None, :] + cf["hc"][:, None]
                ).ravel()
                blob[p, o["wyc"] : o["wyc"] + P * YY] = (
                    cf["wy"][:, None] * ys[None, :] + cf["wc"][:, None]
                ).ravel()
                blob[p, o["hist"] : o["hist"] + YY * V] = (
                    H[g * YY : (g + 1) * YY, :]).ravel()
        for s in range(len(plist), npoly):
            blob[s * G : (s + 1) * G, o["hyc"] : o["hyc"] + P * YY] = 1e3

        blob[:, o["dv"] : o["dv"] + hv[b]] = 1.0

        in_maps.append({"blob": blob.astype(np.float32)})

    res = run_bass_kernel_spmd(nc, in_maps, core_ids=list(range(NCORES)),
                               trace=_trace)
    LAST_RESULTS = res
    # device layout is [H, D*W]; transpose to [D, H, W] while combining
    parts = [r["vox"].reshape(V, V, V).transpose(1, 0, 2)
             for r in res.results]
    out = np.stack([np.maximum(parts[2 * b], parts[2 * b + 1])
                    for b in range(B)])
    return np.ascontiguousarray(out).astype(np.float32)



# revision 7
# speedup vs baseline: 1.5287x; 1.5287x over previous
"""Trainium2 Bass kernel for DifferentiableExtrusion (v2 design).

voxels[b,d,h,w] = depth_mask[b,d] * max_n(valid_n * sigmoid(-100*sdf_n(h,w)))
B=4, N=32 polygons (P=16 vertices), V=128 grid, D=128.

Sharding: 8 cores = (b, row-half). Each core computes ALL valid polygons of
batch b over HALF the grid rows (64 rows), so no cross-core combine is
needed: each core locally max-reduces over its polygon slots and writes its
own [D, 64, W] block of the output (bf16; host converts to f32).

Per-core layout: 128 partitions = S poly slots x 6 row-chunks of YY=11 rows
(chunk bases ch*11 cover local rows 0..65; rows 64,65 are computed but
discarded). Free dim = 11*128 = 1408 pixels.

Per edge e (16 iterations):
  - PE (fp32r matmuls, K=3 against a [x; j; 1] moving tile): h and w linear
    forms into PSUM. Per-partition coefficients come from host-packed
    stationary tiles (the row base y0 is folded into the constant term).
    Filler matmuls keep the PE p-state ramped.
  - DVE: one custom fused op per FD chunk: d2 = relu(|h| - khalf)^2 + w^2
    (f32 PSUM in -> bf16 out). The custom op is registered at import time
    via the documented dve_ops extension point.
  - Pool (gpsimd): macc = min(macc, d2).

Inside test: host quantizes edge/row crossings exactly like the reference
(bit-for-bit f32 compares) and emits a per-pixel +-1 multiplier tile; one
Pool mult-scan turns it into the crossing-parity sign sgn (+1 outside).

Tail: r = sqrt(macc) [ACT], rs = r*sgn [DVE], sig = sigmoid(-100*rs) [ACT],
tree-max over poly slots (strided partition DMAs + DVE max, overlapping
free-dim halves), extrusion staged[p=(dhalf,h), (d',w)] = comb*depth [DVE],
partition-aligned bf16 output DMAs.
"""

import numpy as np

import concourse.bacc as bacc
import concourse.tile as tile
from concourse import mybir
from concourse import dve_ops
from concourse.dve_spec import (Spec, Src0, Src1, C0, Zero, Bin, maxx, sq,
                                lower, _has_src1, AluOp as DAlu)
from concourse.dve_uop import DveOpSpec
from concourse.bass_utils import run_bass_kernel_spmd
from concourse.tile_rust import add_dep_helper

V = 128
P = 16
HALF = 64          # grid rows per core
YY = 11            # rows per partition chunk
NCH = 6            # chunks per polygon (6*11 = 66 >= 64)
FD = YY * V        # 1408 free elements per partition
SHARP = 100.0
EPS = 1e-8
NCORES = 8

F32 = mybir.dt.float32
F32R = mybir.dt.float32r
BF16 = mybir.dt.bfloat16
AF = mybir.ActivationFunctionType
OP = mybir.AluOpType

# FD chunking for PSUM banks (each chunk one 2KB bank; fp32r needs >= 256)
CHUNKS = [(0, 512), (512, 512), (1024, 384)]

# ----------------------------------------------------------------------------
# Custom DVE op: d2 = relu(|h| - c)^2 + w^2  in one instruction
# ----------------------------------------------------------------------------


def _register_d2_op():
    # d2 = relu(|h| - c)^2 + wsq, with h in PSUM and wsq (= w^2, squared on
    # the Activation engine) in SBUF — only one PSUM source is HW-legal.
    name = "EDGE_NEGD2_ANT"
    if name in dve_ops._SUB_OPCODE_FOR_NAME:
        for op in dve_ops.OPS:
            if op.name == name:
                return op
    spec = Spec(
        body=(Zero - sq(maxx(Bin(DAlu.ABSOLUTE_DIFF, Src0, Zero) - C0, Zero)))
        - Src1,
        reference=lambda in0, in1, s0, s1, imm2:
            -(np.maximum(np.abs(in0) - s0, 0.0, dtype=np.float32) ** 2) - in1,
    )
    row = max(dve_ops._SUB_OPCODE_FOR_NAME.values()) + 1
    assert row < 0x20, "no free custom-DVE opcode rows"
    shas = {}
    for ver in ("v3", "v4"):
        uops = lower(spec, ver=ver)
        shas[ver] = DveOpSpec(name=name, opcode=row, uops=uops,
                              rd1_en=_has_src1(spec)).sha(ver)
    op = dve_ops.DveOp(name, spec, subdim=False, uops_sha=shas)
    dve_ops._SUB_OPCODE_FOR_NAME[name] = row
    dve_ops.OPS.append(op)
    dve_ops.CUSTOM_DVE_SPECS[name] = spec
    return op


EDGE_D2 = _register_d2_op()

# ----------------------------------------------------------------------------
# Device module
# ----------------------------------------------------------------------------


def _build_device(S):
    nc = bacc.Bacc()
    mov = nc.declare_dram_parameter("mov", [3, FD], F32R, isOutput=False)
    wst = nc.declare_dram_parameter("wst", [3, P * 2 * 128], F32R, isOutput=False)
    ckh = nc.declare_dram_parameter("ckh", [128, P], F32, isOutput=False)
    cmul = nc.declare_dram_parameter("cmul", [128, FD], BF16, isOutput=False)
    dvv = nc.declare_dram_parameter("dvv", [128, HALF], BF16, isOutput=False)
    vox = nc.declare_dram_parameter("vox", [128, HALF * V], BF16, isOutput=True)

    with tile.TileContext(nc) as tc:
        with (
            tc.tile_pool(name="const", bufs=1) as cpool,
            tc.tile_pool(name="work", bufs=2) as wpool,
            tc.tile_pool(name="acc", bufs=1) as apool,
            tc.tile_pool(name="ps", bufs=1, space="PSUM") as ppool,
        ):
            s_mov = cpool.tile([3, FD], F32R, name="s_mov")
            s_wst = cpool.tile([3, P * 2 * 128], F32R, name="s_wst")
            s_ckh = cpool.tile([128, P], F32, name="s_ckh")
            s_cmul = cpool.tile([128, FD], BF16, name="s_cmul")
            s_dvv = cpool.tile([128, HALF], BF16, name="s_dvv")
            nc.sync.dma_start(out=s_mov, in_=mov[:, :])
            nc.sync.dma_start(out=s_wst, in_=wst[:, :])
            nc.sync.dma_start(out=s_ckh, in_=ckh[:, :])
            nc.sync.dma_start(out=s_cmul, in_=cmul[:, :])
            nc.sync.dma_start(out=s_dvv, in_=dvv[:, :])

            macc = apool.tile([128, FD], BF16, name="macc")
            sgn = apool.tile([128, FD], BF16, name="sgn")
            nc.gpsimd.memset(macc, -1e9)
            # crossing-parity sign via one multiplicative scan
            nc.vector.tensor_tensor_scan(sgn, s_cmul, s_cmul, 1.0,
                                         OP.mult, OP.bypass)

            ph = [ppool.tile([128, 512], F32, name=f"ph{t}") for t in range(3)]
            pw = [ppool.tile([128, 512], F32, name=f"pw{t}") for t in range(3)]
            pdum = [ppool.tile([128, 512], F32, name=f"pd{t}") for t in range(2)]

            def dummy_mm(i):
                nc.tensor.matmul(pdum[i % 2], s_wst[:, 0:128],
                                 s_mov[:, 0:512], start=True, stop=True)

            for i in range(8):
                dummy_mm(i)

            ndum = 0
            for e in range(P):
                d2 = wpool.tile([128, FD], BF16, tag="d2", name="d2")
                wsq = wpool.tile([128, FD], BF16, tag="wsq", name="wsq")
                for t, (o, ln) in enumerate(CHUNKS):
                    wh = s_wst[:, e * 256 : e * 256 + 128]
                    ww = s_wst[:, e * 256 + 128 : e * 256 + 256]
                    nc.tensor.matmul(ph[t][:, :ln], wh, s_mov[:, o : o + ln],
                                     start=True, stop=True)
                    nc.tensor.matmul(pw[t][:, :ln], ww, s_mov[:, o : o + ln],
                                     start=True, stop=True)
                for t, (o, ln) in enumerate(CHUNKS):
                    nc.scalar.activation(wsq[:, o : o + ln], pw[t][:, :ln],
                                         AF.Square)
                    nc.vector._custom_dve(
                        EDGE_D2, out=d2[:, o : o + ln],
                        in0=ph[t][:, :ln], in1=wsq[:, o : o + ln],
                        s0=s_ckh[:, e : e + 1])
                nc.vector.tensor_tensor(macc, macc, d2, OP.max)
                # keep the PE p-state ramped between edges
                for _ in range(4):
                    dummy_mm(ndum)
                    ndum += 1

            # mask = sigmoid(-100 * sgn * sqrt(-macc))  (macc holds -d2)
            nc.scalar.activation(macc, macc, AF.Sqrt, scale=-1.0)
            nc.gpsimd.tensor_tensor(macc, macc, sgn, OP.mult)
            sig = apool.tile([128, FD], BF16, name="sig")
            nc.scalar.activation(sig, macc, AF.Sigmoid, scale=-SHARP)

            # tree-max over poly slots (partitions are slot-major, G=6)
            HFD = FD // 2
            prev = [None, None]
            scur = S
            while scur > 1:
                k = scur // 2
                npar = k * NCH
                src0 = (scur - k) * NCH
                tmp = wpool.tile([npar, FD], BF16, tag="tree", name="tmp")
                for xh in range(2):
                    sl = slice(xh * HFD, (xh + 1) * HFD)
                    d_i = nc.sync.dma_start(out=tmp[:npar, sl],
                                            in_=sig[src0 : src0 + npar, sl])
                    if prev[xh] is not None:
                        add_dep_helper(d_i.ins, prev[xh].ins,
                                       reason="tree round reads prior max")
                    t_i = nc.vector.tensor_tensor(
                        sig[:npar, sl], sig[:npar, sl], tmp[:npar, sl], OP.max)
                    add_dep_helper(t_i.ins, d_i.ins, reason="tree max after dma")
                    prev[xh] = t_i
                scur -= k

            # comb66[0:64] = the 64 local grid rows; replicate into comb2
            comb66 = apool.tile([66, V], BF16, name="comb66")
            c_i = nc.sync.dma_start(out=comb66, in_=sig[:NCH])
            for xh in range(2):
                add_dep_helper(c_i.ins, prev[xh].ins,
                               reason="comb reads final tree max")
            comb2 = apool.tile([128, V], BF16, name="comb2")
            r0_i = nc.sync.dma_start(out=comb2[0:64], in_=comb66[0:64])
            add_dep_helper(r0_i.ins, c_i.ins, reason="comb replicate")
            r1_i = nc.sync.dma_start(out=comb2[64:128], in_=comb66[0:64])
            add_dep_helper(r1_i.ins, c_i.ins, reason="comb replicate")

            # extrusion: staged[p=(dhalf,h), (d16,w)] = comb2[p,w]*dvv[p,d]
            HD = 16
            for i in range(4):
                stg = wpool.tile([128, HD, V], BF16, tag="stg", name="stg")
                m_i = nc.gpsimd.tensor_tensor(
                    stg,
                    comb2.unsqueeze(1).broadcast_to([128, HD, V]),
                    s_dvv[:, i * HD : (i + 1) * HD].unsqueeze(2).broadcast_to(
                        [128, HD, V]),
                    OP.mult)
                add_dep_helper(m_i.ins, r0_i.ins, reason="staging reads comb2")
                add_dep_helper(m_i.ins, r1_i.ins, reason="staging reads comb2")
                nc.sync.dma_start(out=vox[:, i * HD * V : (i + 1) * HD * V],
                                  in_=stg)

    nc.compile()
    return nc


_NC_CACHE = {}


def _get_nc(S):
    if S not in _NC_CACHE:
        _NC_CACHE[S] = _build_device(S)
    return _NC_CACHE[S]


# ----------------------------------------------------------------------------
# Host-side: polygon -> per-edge linear-form coefficients + crossing parity
# ----------------------------------------------------------------------------


def _poly_coeffs(poly):
    vmask = poly.sum(axis=1) != 0.0
    K = int(vmask.sum())
    order = np.argsort((~vmask).astype(np.int32), kind="stable")
    pv = poly[order].astype(np.float64)
    idx = np.arange(P)
    nxt = np.where(idx == K - 1, 0, idx + 1)
    v0 = pv
    v1 = pv[nxt]
    valid_e = idx < K if K >= 3 else np.zeros(P, bool)

    ex = v1[:, 0] - v0[:, 0]
    ey = v1[:, 1] - v0[:, 1]
    s2 = ex * ex + ey * ey + EPS
    k = np.sqrt(s2)

    hx = -ex / k
    hy = -ey / k
    hc = (v0[:, 0] * ex + v0[:, 1] * ey) / k + k / 2.0
    wx = -ey / k
    wy = ex / k
    wc = (ey * v0[:, 0] - ex * v0[:, 1]) / k

    hx = np.where(valid_e, hx, 0.0)
    hy = np.where(valid_e, hy, 0.0)
    hc = np.where(valid_e, hc, 1e3)
    wx = np.where(valid_e, wx, 0.0)
    wy = np.where(valid_e, wy, 0.0)
    wc = np.where(valid_e, wc, 0.0)
    khalf = np.where(valid_e, k / 2.0, 0.0)

    # crossing columns, f32 ops mirroring the reference bit-for-bit:
    # thr[e, y] = #{grid columns j with inter_x > x_j}; 0 when !y_crosses
    x32 = np.arange(V, dtype=np.float32) / np.float32(V - 1)
    y32 = x32
    x0 = v0[:, 0].astype(np.float32)[:, None]
    y0 = v0[:, 1].astype(np.float32)[:, None]
    x1 = v1[:, 0].astype(np.float32)[:, None]
    y1 = v1[:, 1].astype(np.float32)[:, None]
    yrow = y32[None, :]
    yc = ((y0 <= yrow) & (y1 > yrow)) | ((y1 <= yrow) & (y0 > yrow))
    t = (yrow - y0) / (y1 - y0 + np.float32(EPS))
    ix = x0 + (x1 - x0) * t                                   # (P, V) f32
    yc = yc & valid_e[:, None]
    thr = (ix[:, :, None] > x32[None, None, :]).sum(axis=2)   # (P, V) ints
    thr = np.where(yc, thr, 0)

    return dict(hx=hx, hy=hy, hc=hc, wx=wx, wy=wy, wc=wc, khalf=khalf,
                thr=thr)


def _parity_tables(thr):
    """Per-row crossing-parity histogram for one polygon.
    Returns (pm, rowpar): pm[y, j] = (-1)^{Htilde[y, j]} with
    Htilde[y, 0] = #{thr >= 1}, Htilde[y, j>=1] = #{thr == j}; the running
    product of row y's prefix has the parity of pixel (y, j)'s crossing
    count. rowpar[y] = parity of the whole row's Htilde sum."""
    Ht = np.zeros((V, V), np.int64)
    for y in range(V):
        th = thr[:, y]
        hist = np.bincount(th[(th >= 1) & (th <= V - 1)], minlength=V)
        Ht[y, 1:] = hist[1:]
        Ht[y, 0] = int((th >= 1).sum())
    pm = np.where(Ht % 2 == 1, -1.0, 1.0).astype(np.float32)
    rowpar = (Ht.sum(axis=1) % 2).astype(np.int64)
    return pm, rowpar


# ----------------------------------------------------------------------------
# Host entry point
# ----------------------------------------------------------------------------

LAST_RESULTS = None


def kernel(polygons, attributes, validity_scores, _trace=False):
    global LAST_RESULTS
    polygons = np.asarray(polygons)
    attributes = np.asarray(attributes)
    validity_scores = np.asarray(validity_scores)
    B, N, _, _ = polygons.shape
    assert (B, N) == (4, 32)

    valid_lists = [[n for n in range(N) if validity_scores[b, n] >= 0.5]
                   for b in range(B)]
    S = max(2, max(len(v) for v in valid_lists))
    assert S * NCH <= 128, f"too many valid polygons: {S}"
    nc = _get_nc(S)

    norm = np.clip(attributes[:, 0].astype(np.float32), 0.0, 1.0)
    hv = np.clip(np.rint(norm * np.float32(V)).astype(np.int32), 1, V)

    # per-(b, poly) precompute shared by both half-cores
    coeffs = {}
    parity = {}
    for b in range(B):
        for n in valid_lists[b]:
            cf = _poly_coeffs(np.asarray(polygons[b, n], np.float32))
            coeffs[(b, n)] = cf
            parity[(b, n)] = _parity_tables(cf["thr"])

    # moving tile: rows (x, j, 1) in free order f = j*V + c
    x32 = np.arange(V, dtype=np.float32) / np.float32(V - 1)
    movt = np.zeros((3, FD), np.float32)
    movt[0] = np.tile(x32, YY)
    movt[1] = np.repeat(np.arange(YY, dtype=np.float32), V)
    movt[2] = 1.0

    in_maps = []
    for c in range(NCORES):
        b, half = c // 2, c % 2
        plist = valid_lists[b]

        wstv = np.zeros((3, P * 2 * 128), np.float64)
        ckhv = np.zeros((128, P), np.float64)
        cmulv = np.ones((128, FD), np.float32)
        for p in range(128):
            s, ch = p // NCH, p % NCH
            if s < len(plist):
                cf = coeffs[(b, plist[s])]
                y0 = (half * HALF + ch * YY) / 127.0
                for e in range(P):
                    o = e * 256
                    wstv[0, o + p] = cf["hx"][e]
                    wstv[1, o + p] = cf["hy"][e] / 127.0
                    wstv[2, o + p] = cf["hy"][e] * y0 + cf["hc"][e]
                    wstv[0, o + 128 + p] = cf["wx"][e]
                    wstv[1, o + 128 + p] = cf["wy"][e] / 127.0
                    wstv[2, o + 128 + p] = cf["wy"][e] * y0 + cf["wc"][e]
                ckhv[p] = cf["khalf"]
                pm, rowpar = parity[(b, plist[s])]
                run = 0
                for j in range(YY):
                    row = half * HALF + ch * YY + j
                    if row >= V:
                        break
                    cmulv[p, j * V : (j + 1) * V] = pm[row]
                    if j > 0 and run % 2 == 1:
                        cmulv[p, j * V] = -cmulv[p, j * V]
                        run = 0
                    run += int(rowpar[row])
            else:
                for e in range(P):
                    wstv[2, e * 256 + p] = 1e3

        dvvv = np.zeros((128, HALF), np.float32)
        dmask = (np.arange(V) < hv[b]).astype(np.float32)
        for p in range(128):
            dh = p // 64
            dvvv[p] = dmask[dh * 64 : (dh + 1) * 64]

        import ml_dtypes
        in_maps.append({
            "mov": movt.astype(np.float32),
            "wst": wstv.astype(np.float32),
            "ckh": ckhv.astype(np.float32),
            "cmul": cmulv.astype(ml_dtypes.bfloat16),
            "dvv": dvvv.astype(ml_dtypes.bfloat16),
        })

    res = run_bass_kernel_spmd(nc, in_maps, core_ids=list(range(NCORES)),
                               trace=_trace)
    LAST_RESULTS = res

    out = np.zeros((B, V, V, V), np.float32)
    for c in range(NCORES):
        b, half = c // 2, c % 2
        a = np.asarray(res.results[c]["vox"]).astype(np.float32)
        a = a.reshape(2, 64, HALF, V)            # [dhalf, h, d', w]
        out[b, :, half * HALF : (half + 1) * HALF, :] = (
            a.transpose(0, 2, 1, 3).reshape(V, HALF, V))
    return np.ascontiguousarray(out)


# revision 8
# speedup vs baseline: 1.8373x; 1.2019x over previous
"""Trainium2 Bass kernel for DifferentiableExtrusion (v2 design).

voxels[b,d,h,w] = depth_mask[b,d] * max_n(valid_n * sigmoid(-100*sdf_n(h,w)))
B=4, N=32 polygons (P=16 vertices), V=128 grid, D=128.

Sharding: 8 cores = (b, row-half). Each core computes ALL valid polygons of
batch b over HALF the grid rows (64 rows), so no cross-core combine is
needed: each core locally max-reduces over its polygon slots and writes its
own [D, 64, W] block of the output (bf16; host converts to f32).

Per-core layout: 128 partitions = S poly slots x 6 row-chunks of YY=11 rows
(chunk bases ch*11 cover local rows 0..65; rows 64,65 are computed but
discarded). Free dim = 11*128 = 1408 pixels.

Per edge e (16 iterations):
  - PE (fp32r matmuls, K=3 against a [x; j; 1] moving tile): h and w linear
    forms into PSUM. Per-partition coefficients come from host-packed
    stationary tiles (the row base y0 is folded into the constant term).
    Filler matmuls keep the PE p-state ramped.
  - DVE: one custom fused op per FD chunk: d2 = relu(|h| - khalf)^2 + w^2
    (f32 PSUM in -> bf16 out). The custom op is registered at import time
    via the documented dve_ops extension point.
  - Pool (gpsimd): macc = min(macc, d2).

Inside test: host quantizes edge/row crossings exactly like the reference
(bit-for-bit f32 compares) and emits a per-pixel +-1 multiplier tile; one
Pool mult-scan turns it into the crossing-parity sign sgn (+1 outside).

Tail: r = sqrt(macc) [ACT], rs = r*sgn [DVE], sig = sigmoid(-100*rs) [ACT],
tree-max over poly slots (strided partition DMAs + DVE max, overlapping
free-dim halves), extrusion staged[p=(dhalf,h), (d',w)] = comb*depth [DVE],
partition-aligned bf16 output DMAs.
"""

import numpy as np

import concourse.bacc as bacc
import concourse.tile as tile
from concourse import mybir
from concourse import dve_ops
from concourse.dve_spec import (Spec, Src0, Src1, C0, Zero, Bin, maxx, sq,
                                lower, _has_src1, AluOp as DAlu)
from concourse.dve_uop import DveOpSpec
from concourse.bass_utils import run_bass_kernel_spmd
from concourse.tile_rust import add_dep_helper

V = 128
P = 16
HALF = 64          # grid rows per core
YY = 11            # rows per partition chunk
NCH = 6            # chunks per polygon (6*11 = 66 >= 64)
FD = YY * V        # 1408 free elements per partition
SHARP = 100.0
EPS = 1e-8
NCORES = 8

F32 = mybir.dt.float32
F32R = mybir.dt.float32r
BF16 = mybir.dt.bfloat16
AF = mybir.ActivationFunctionType
OP = mybir.AluOpType

# FD chunking for PSUM banks (each chunk one 2KB bank; fp32r needs >= 256)
CHUNKS = [(0, 512), (512, 512), (1024, 384)]

# ----------------------------------------------------------------------------
# Custom DVE op: d2 = relu(|h| - c)^2 + w^2  in one instruction
# ----------------------------------------------------------------------------


def _register_d2_op():
    # d2 = relu(|h| - c)^2 + wsq, with h in PSUM and wsq (= w^2, squared on
    # the Activation engine) in SBUF — only one PSUM source is HW-legal.
    name = "EDGE_NEGD2_ANT"
    if name in dve_ops._SUB_OPCODE_FOR_NAME:
        for op in dve_ops.OPS:
            if op.name == name:
                return op
    spec = Spec(
        body=(Zero - sq(maxx(Bin(DAlu.ABSOLUTE_DIFF, Src0, Zero) - C0, Zero)))
        - Src1,
        reference=lambda in0, in1, s0, s1, imm2:
            -(np.maximum(np.abs(in0) - s0, 0.0, dtype=np.float32) ** 2) - in1,
    )
    row = max(dve_ops._SUB_OPCODE_FOR_NAME.values()) + 1
    assert row < 0x20, "no free custom-DVE opcode rows"
    shas = {}
    for ver in ("v3", "v4"):
        uops = lower(spec, ver=ver)
        shas[ver] = DveOpSpec(name=name, opcode=row, uops=uops,
                              rd1_en=_has_src1(spec)).sha(ver)
    op = dve_ops.DveOp(name, spec, subdim=False, uops_sha=shas)
    dve_ops._SUB_OPCODE_FOR_NAME[name] = row
    dve_ops.OPS.append(op)
    dve_ops.CUSTOM_DVE_SPECS[name] = spec
    return op


EDGE_D2 = _register_d2_op()

# ----------------------------------------------------------------------------
# Device module
# ----------------------------------------------------------------------------


def _build_device(S):
    nc = bacc.Bacc()
    mov = nc.declare_dram_parameter("mov", [3, FD], F32R, isOutput=False)
    wst = nc.declare_dram_parameter("wst", [3, P * 2 * 128], F32R, isOutput=False)
    ckh = nc.declare_dram_parameter("ckh", [128, P], F32, isOutput=False)
    cmul = nc.declare_dram_parameter("cmul", [128, FD], BF16, isOutput=False)
    dvv = nc.declare_dram_parameter("dvv", [128, HALF], BF16, isOutput=False)
    vox = nc.declare_dram_parameter("vox", [128, HALF * V], BF16, isOutput=True)

    with tile.TileContext(nc) as tc:
        with (
            tc.tile_pool(name="const", bufs=1) as cpool,
            tc.tile_pool(name="work", bufs=2) as wpool,
            tc.tile_pool(name="acc", bufs=1) as apool,
            tc.tile_pool(name="ps", bufs=1, space="PSUM") as ppool,
        ):
            s_mov = cpool.tile([3, FD], F32R, name="s_mov")
            s_wst = cpool.tile([3, P * 2 * 128], F32R, name="s_wst")
            s_ckh = cpool.tile([128, P], F32, name="s_ckh")
            s_cmul = cpool.tile([128, FD], BF16, name="s_cmul")
            s_dvv = cpool.tile([128, HALF], BF16, name="s_dvv")
            nc.sync.dma_start(out=s_mov, in_=mov[:, :])
            nc.sync.dma_start(out=s_wst, in_=wst[:, :])
            nc.sync.dma_start(out=s_ckh, in_=ckh[:, :])
            nc.sync.dma_start(out=s_cmul, in_=cmul[:, :])
            nc.sync.dma_start(out=s_dvv, in_=dvv[:, :])

            macc = apool.tile([128, FD], BF16, name="macc")
            sgn = apool.tile([128, FD], BF16, name="sgn")
            nc.gpsimd.memset(macc, -1e9)
            # crossing-parity sign via one multiplicative scan
            nc.vector.tensor_tensor_scan(sgn, s_cmul, s_cmul, 1.0,
                                         OP.mult, OP.bypass)

            ph = [ppool.tile([128, 512], F32, name=f"ph{t}") for t in range(3)]
            pw = [ppool.tile([128, 512], F32, name=f"pw{t}") for t in range(3)]

            # warm the activation-function tables while inputs load
            warm = apool.tile([128, 1], BF16, name="warm")
            nc.vector.memset(warm, 1.0)
            nc.scalar.activation(warm, warm, AF.Square)
            nc.scalar.activation(warm, warm, AF.Sqrt)
            nc.scalar.activation(warm, warm, AF.Sigmoid)
            for e in range(P):
                d2 = wpool.tile([128, FD], BF16, tag="d2", name="d2")
                wsq = wpool.tile([128, FD], BF16, tag="wsq", name="wsq")
                for t, (o, ln) in enumerate(CHUNKS):
                    wh = s_wst[:, e * 256 : e * 256 + 128]
                    ww = s_wst[:, e * 256 + 128 : e * 256 + 256]
                    nc.tensor.matmul(ph[t][:, :ln], wh, s_mov[:, o : o + ln],
                                     start=True, stop=True)
                    nc.tensor.matmul(pw[t][:, :ln], ww, s_mov[:, o : o + ln],
                                     start=True, stop=True)
                for t, (o, ln) in enumerate(CHUNKS):
                    nc.scalar.activation(wsq[:, o : o + ln], pw[t][:, :ln],
                                         AF.Square)
                    nc.vector._custom_dve(
                        EDGE_D2, out=d2[:, o : o + ln],
                        in0=ph[t][:, :ln], in1=wsq[:, o : o + ln],
                        s0=s_ckh[:, e : e + 1])
                nc.vector.tensor_tensor(macc, macc, d2, OP.max)

            # mask = sigmoid(-100 * sgn * sqrt(-macc))  (macc holds -d2)
            nc.scalar.activation(macc, macc, AF.Sqrt, scale=-1.0)
            nc.vector.tensor_tensor(macc, macc, sgn, OP.mult)
            sig = apool.tile([128, FD], BF16, name="sig")
            nc.scalar.activation(sig, macc, AF.Sigmoid, scale=-SHARP)

            # tree-max over poly slots (partitions are slot-major, G=6)
            HFD = FD // 2
            prev = [None, None]
            scur = S
            while scur > 1:
                k = scur // 2
                npar = k * NCH
                src0 = (scur - k) * NCH
                tmp = wpool.tile([npar, FD], BF16, tag="tree", name="tmp")
                for xh in range(2):
                    sl = slice(xh * HFD, (xh + 1) * HFD)
                    d_i = nc.sync.dma_start(out=tmp[:npar, sl],
                                            in_=sig[src0 : src0 + npar, sl])
                    if prev[xh] is not None:
                        add_dep_helper(d_i.ins, prev[xh].ins,
                                       reason="tree round reads prior max")
                    t_i = nc.vector.tensor_tensor(
                        sig[:npar, sl], sig[:npar, sl], tmp[:npar, sl], OP.max)
                    add_dep_helper(t_i.ins, d_i.ins, reason="tree max after dma")
                    prev[xh] = t_i
                scur -= k

            # comb66[0:64] = the 64 local grid rows; replicate into comb2
            comb66 = apool.tile([66, V], BF16, name="comb66")
            c_i = nc.sync.dma_start(out=comb66, in_=sig[:NCH])
            for xh in range(2):
                add_dep_helper(c_i.ins, prev[xh].ins,
                               reason="comb reads final tree max")
            comb2 = apool.tile([128, V], BF16, name="comb2")
            r0_i = nc.sync.dma_start(out=comb2[0:64], in_=comb66[0:64])
            add_dep_helper(r0_i.ins, c_i.ins, reason="comb replicate")
            r1_i = nc.sync.dma_start(out=comb2[64:128], in_=comb66[0:64])
            add_dep_helper(r1_i.ins, c_i.ins, reason="comb replicate")

            # extrusion: staged[p=(dhalf,h), (d16,w)] = comb2[p,w]*dvv[p,d]
            HD = 16
            for i in range(4):
                stg = wpool.tile([128, HD, V], BF16, tag="stg", name="stg")
                eng = nc.gpsimd if i == 3 else nc.vector
                m_i = eng.tensor_tensor(
                    stg,
                    comb2.unsqueeze(1).broadcast_to([128, HD, V]),
                    s_dvv[:, i * HD : (i + 1) * HD].unsqueeze(2).broadcast_to(
                        [128, HD, V]),
                    OP.mult)
                add_dep_helper(m_i.ins, r0_i.ins, reason="staging reads comb2")
                add_dep_helper(m_i.ins, r1_i.ins, reason="staging reads comb2")
                nc.sync.dma_start(out=vox[:, i * HD * V : (i + 1) * HD * V],
                                  in_=stg)

    nc.compile()
    return nc


_NC_CACHE = {}


def _get_nc(S):
    if S not in _NC_CACHE:
        _NC_CACHE[S] = _build_device(S)
    return _NC_CACHE[S]


# ----------------------------------------------------------------------------
# Host-side: polygon -> per-edge linear-form coefficients + crossing parity
# ----------------------------------------------------------------------------


def _poly_coeffs(poly):
    vmask = poly.sum(axis=1) != 0.0
    K = int(vmask.sum())
    order = np.argsort((~vmask).astype(np.int32), kind="stable")
    pv = poly[order].astype(np.float64)
    idx = np.arange(P)
    nxt = np.where(idx == K - 1, 0, idx + 1)
    v0 = pv
    v1 = pv[nxt]
    valid_e = idx < K if K >= 3 else np.zeros(P, bool)

    ex = v1[:, 0] - v0[:, 0]
    ey = v1[:, 1] - v0[:, 1]
    s2 = ex * ex + ey * ey + EPS
    k = np.sqrt(s2)

    hx = -ex / k
    hy = -ey / k
    hc = (v0[:, 0] * ex + v0[:, 1] * ey) / k + k / 2.0
    wx = -ey / k
    wy = ex / k
    wc = (ey * v0[:, 0] - ex * v0[:, 1]) / k

    hx = np.where(valid_e, hx, 0.0)
    hy = np.where(valid_e, hy, 0.0)
    hc = np.where(valid_e, hc, 1e3)
    wx = np.where(valid_e, wx, 0.0)
    wy = np.where(valid_e, wy, 0.0)
    wc = np.where(valid_e, wc, 0.0)
    khalf = np.where(valid_e, k / 2.0, 0.0)

    # crossing columns, f32 ops mirroring the reference bit-for-bit:
    # thr[e, y] = #{grid columns j with inter_x > x_j}; 0 when !y_crosses
    x32 = np.arange(V, dtype=np.float32) / np.float32(V - 1)
    y32 = x32
    x0 = v0[:, 0].astype(np.float32)[:, None]
    y0 = v0[:, 1].astype(np.float32)[:, None]
    x1 = v1[:, 0].astype(np.float32)[:, None]
    y1 = v1[:, 1].astype(np.float32)[:, None]
    yrow = y32[None, :]
    yc = ((y0 <= yrow) & (y1 > yrow)) | ((y1 <= yrow) & (y0 > yrow))
    t = (yrow - y0) / (y1 - y0 + np.float32(EPS))
    ix = x0 + (x1 - x0) * t                                   # (P, V) f32
    yc = yc & valid_e[:, None]
    thr = (ix[:, :, None] > x32[None, None, :]).sum(axis=2)   # (P, V) ints
    thr = np.where(yc, thr, 0)

    return dict(hx=hx, hy=hy, hc=hc, wx=wx, wy=wy, wc=wc, khalf=khalf,
                thr=thr)


def _parity_tables(thr):
    """Per-row crossing-parity histogram for one polygon.
    Returns (pm, rowpar): pm[y, j] = (-1)^{Htilde[y, j]} with
    Htilde[y, 0] = #{thr >= 1}, Htilde[y, j>=1] = #{thr == j}; the running
    product of row y's prefix has the parity of pixel (y, j)'s crossing
    count. rowpar[y] = parity of the whole row's Htilde sum."""
    Ht = np.zeros((V, V), np.int64)
    for y in range(V):
        th = thr[:, y]
        hist = np.bincount(th[(th >= 1) & (th <= V - 1)], minlength=V)
        Ht[y, 1:] = hist[1:]
        Ht[y, 0] = int((th >= 1).sum())
    pm = np.where(Ht % 2 == 1, -1.0, 1.0).astype(np.float32)
    rowpar = (Ht.sum(axis=1) % 2).astype(np.int64)
    return pm, rowpar


# ----------------------------------------------------------------------------
# Host entry point
# ----------------------------------------------------------------------------

LAST_RESULTS = None


def kernel(polygons, attributes, validity_scores, _trace=False):
    global LAST_RESULTS
    polygons = np.asarray(polygons)
    attributes = np.asarray(attributes)
    validity_scores = np.asarray(validity_scores)
    B, N, _, _ = polygons.shape
    assert (B, N) == (4, 32)

    valid_lists = [[n for n in range(N) if validity_scores[b, n] >= 0.5]
                   for b in range(B)]
    S = max(2, max(len(v) for v in valid_lists))
    assert S * NCH <= 128, f"too many valid polygons: {S}"
    nc = _get_nc(S)

    norm = np.clip(attributes[:, 0].astype(np.float32), 0.0, 1.0)
    hv = np.clip(np.rint(norm * np.float32(V)).astype(np.int32), 1, V)

    # per-(b, poly) precompute shared by both half-cores
    coeffs = {}
    parity = {}
    for b in range(B):
        for n in valid_lists[b]:
            cf = _poly_coeffs(np.asarray(polygons[b, n], np.float32))
            coeffs[(b, n)] = cf
            parity[(b, n)] = _parity_tables(cf["thr"])

    # moving tile: rows (x, j, 1) in free order f = j*V + c
    x32 = np.arange(V, dtype=np.float32) / np.float32(V - 1)
    movt = np.zeros((3, FD), np.float32)
    movt[0] = np.tile(x32, YY)
    movt[1] = np.repeat(np.arange(YY, dtype=np.float32), V)
    movt[2] = 1.0

    in_maps = []
    for c in range(NCORES):
        b, half = c // 2, c % 2
        plist = valid_lists[b]

        wstv = np.zeros((3, P * 2 * 128), np.float64)
        ckhv = np.zeros((128, P), np.float64)
        cmulv = np.ones((128, FD), np.float32)
        for p in range(128):
            s, ch = p // NCH, p % NCH
            if s < len(plist):
                cf = coeffs[(b, plist[s])]
                y0 = (half * HALF + ch * YY) / 127.0
                for e in range(P):
                    o = e * 256
                    wstv[0, o + p] = cf["hx"][e]
                    wstv[1, o + p] = cf["hy"][e] / 127.0
                    wstv[2, o + p] = cf["hy"][e] * y0 + cf["hc"][e]
                    wstv[0, o + 128 + p] = cf["wx"][e]
                    wstv[1, o + 128 + p] = cf["wy"][e] / 127.0
                    wstv[2, o + 128 + p] = cf["wy"][e] * y0 + cf["wc"][e]
                ckhv[p] = cf["khalf"]
                pm, rowpar = parity[(b, plist[s])]
                run = 0
                for j in range(YY):
                    row = half * HALF + ch * YY + j
                    if row >= V:
                        break
                    cmulv[p, j * V : (j + 1) * V] = pm[row]
                    if j > 0 and run % 2 == 1:
                        cmulv[p, j * V] = -cmulv[p, j * V]
                        run = 0
                    run += int(rowpar[row])
            else:
                for e in range(P):
                    wstv[2, e * 256 + p] = 1e3

        dvvv = np.zeros((128, HALF), np.float32)
        dmask = (np.arange(V) < hv[b]).astype(np.float32)
        for p in range(128):
            dh = p // 64
            dvvv[p] = dmask[dh * 64 : (dh + 1) * 64]

        import ml_dtypes
        in_maps.append({
            "mov": movt.astype(np.float32),
            "wst": wstv.astype(np.float32),
            "ckh": ckhv.astype(np.float32),
            "cmul": cmulv.astype(ml_dtypes.bfloat16),
            "dvv": dvvv.astype(ml_dtypes.bfloat16),
        })

    res = run_bass_kernel_spmd(nc, in_maps, core_ids=list(range(NCORES)),
                               trace=_trace)
    LAST_RESULTS = res

    out = np.zeros((B, V, V, V), np.float32)
    for c in range(NCORES):
        b, half = c // 2, c % 2
        a = np.asarray(res.results[c]["vox"]).astype(np.float32)
        a = a.reshape(2, 64, HALF, V)            # [dhalf, h, d', w]
        out[b, :, half * HALF : (half + 1) * HALF, :] = (
            a.transpose(0, 2, 1, 3).reshape(V, HALF, V))
    return np.ascontiguousarray(out)


# revision 9
# speedup vs baseline: 1.8414x; 1.0023x over previous
"""Trainium2 Bass kernel for DifferentiableExtrusion (v2 design).

voxels[b,d,h,w] = depth_mask[b,d] * max_n(valid_n * sigmoid(-100*sdf_n(h,w)))
B=4, N=32 polygons (P=16 vertices), V=128 grid, D=128.

Sharding: 8 cores = (b, row-half). Each core computes ALL valid polygons of
batch b over HALF the grid rows (64 rows), so no cross-core combine is
needed: each core locally max-reduces over its polygon slots and writes its
own [D, 64, W] block of the output (bf16; host converts to f32).

Per-core layout: 128 partitions = S poly slots x 6 row-chunks of YY=11 rows
(chunk bases ch*11 cover local rows 0..65; rows 64,65 are computed but
discarded). Free dim = 11*128 = 1408 pixels.

Per edge e (16 iterations):
  - PE (fp32r matmuls, K=3 against a [x; j; 1] moving tile): h and w linear
    forms into PSUM. Per-partition coefficients come from host-packed
    stationary tiles (the row base y0 is folded into the constant term).
    Filler matmuls keep the PE p-state ramped.
  - DVE: one custom fused op per FD chunk: d2 = relu(|h| - khalf)^2 + w^2
    (f32 PSUM in -> bf16 out). The custom op is registered at import time
    via the documented dve_ops extension point.
  - Pool (gpsimd): macc = min(macc, d2).

Inside test: host quantizes edge/row crossings exactly like the reference
(bit-for-bit f32 compares) and emits a per-pixel +-1 multiplier tile; one
Pool mult-scan turns it into the crossing-parity sign sgn (+1 outside).

Tail: r = sqrt(macc) [ACT], rs = r*sgn [DVE], sig = sigmoid(-100*rs) [ACT],
tree-max over poly slots (strided partition DMAs + DVE max, overlapping
free-dim halves), extrusion staged[p=(dhalf,h), (d',w)] = comb*depth [DVE],
partition-aligned bf16 output DMAs.
"""

import numpy as np

import concourse.bacc as bacc
import concourse.tile as tile
from concourse import mybir
from concourse import dve_ops
from concourse.dve_spec import (Spec, Src0, Src1, C0, Zero, Bin, maxx, sq,
                                lower, _has_src1, AluOp as DAlu)
from concourse.dve_uop import DveOpSpec
from concourse.bass_utils import run_bass_kernel_spmd
from concourse.tile_rust import add_dep_helper

V = 128
P = 16
HALF = 64          # grid rows per core
YY = 11            # rows per partition chunk
NCH = 6            # chunks per polygon (6*11 = 66 >= 64)
FD = YY * V        # 1408 free elements per partition
SHARP = 100.0
EPS = 1e-8
NCORES = 8

F32 = mybir.dt.float32
F32R = mybir.dt.float32r
BF16 = mybir.dt.bfloat16
AF = mybir.ActivationFunctionType
OP = mybir.AluOpType

# FD chunking for PSUM banks (each chunk one 2KB bank; fp32r needs >= 256)
CHUNKS = [(0, 512), (512, 512), (1024, 384)]

# ----------------------------------------------------------------------------
# Custom DVE op: d2 = relu(|h| - c)^2 + w^2  in one instruction
# ----------------------------------------------------------------------------


def _register_d2_op():
    # d2 = relu(|h| - c)^2 + wsq, with h in PSUM and wsq (= w^2, squared on
    # the Activation engine) in SBUF — only one PSUM source is HW-legal.
    name = "EDGE_NEGD2_ANT"
    if name in dve_ops._SUB_OPCODE_FOR_NAME:
        for op in dve_ops.OPS:
            if op.name == name:
                return op
    spec = Spec(
        body=(Zero - sq(maxx(Bin(DAlu.ABSOLUTE_DIFF, Src0, Zero) - C0, Zero)))
        - Src1,
        reference=lambda in0, in1, s0, s1, imm2:
            -(np.maximum(np.abs(in0) - s0, 0.0, dtype=np.float32) ** 2) - in1,
    )
    row = max(dve_ops._SUB_OPCODE_FOR_NAME.values()) + 1
    assert row < 0x20, "no free custom-DVE opcode rows"
    shas = {}
    for ver in ("v3", "v4"):
        uops = lower(spec, ver=ver)
        shas[ver] = DveOpSpec(name=name, opcode=row, uops=uops,
                              rd1_en=_has_src1(spec)).sha(ver)
    op = dve_ops.DveOp(name, spec, subdim=False, uops_sha=shas)
    dve_ops._SUB_OPCODE_FOR_NAME[name] = row
    dve_ops.OPS.append(op)
    dve_ops.CUSTOM_DVE_SPECS[name] = spec
    return op


EDGE_D2 = _register_d2_op()

# ----------------------------------------------------------------------------
# Device module
# ----------------------------------------------------------------------------


def _build_device(S):
    nc = bacc.Bacc()
    mov = nc.declare_dram_parameter("mov", [3, FD], F32R, isOutput=False)
    wst = nc.declare_dram_parameter("wst", [3, P * 2 * 128], F32R, isOutput=False)
    ckh = nc.declare_dram_parameter("ckh", [128, P], F32, isOutput=False)
    cmul = nc.declare_dram_parameter("cmul", [128, FD], BF16, isOutput=False)
    dvv = nc.declare_dram_parameter("dvv", [128, HALF], BF16, isOutput=False)
    vox = nc.declare_dram_parameter("vox", [128, HALF * V], BF16, isOutput=True)

    with tile.TileContext(nc) as tc:
        with (
            tc.tile_pool(name="const", bufs=1) as cpool,
            tc.tile_pool(name="work", bufs=2) as wpool,
            tc.tile_pool(name="acc", bufs=1) as apool,
            tc.tile_pool(name="ps", bufs=1, space="PSUM") as ppool,
        ):
            s_mov = cpool.tile([3, FD], F32R, name="s_mov")
            s_wst = cpool.tile([3, P * 2 * 128], F32R, name="s_wst")
            s_ckh = cpool.tile([128, P], F32, name="s_ckh")
            s_cmul = cpool.tile([128, FD], BF16, name="s_cmul")
            s_dvv = cpool.tile([128, HALF], BF16, name="s_dvv")
            nc.sync.dma_start(out=s_mov, in_=mov[:, :])
            nc.sync.dma_start(out=s_wst, in_=wst[:, :])
            nc.sync.dma_start(out=s_ckh, in_=ckh[:, :])
            nc.sync.dma_start(out=s_cmul, in_=cmul[:, :])
            nc.sync.dma_start(out=s_dvv, in_=dvv[:, :])

            macc = apool.tile([128, FD], BF16, name="macc")
            sgn = apool.tile([128, FD], BF16, name="sgn")
            nc.gpsimd.memset(macc, -1e9)
            # crossing-parity sign via one multiplicative scan
            nc.vector.tensor_tensor_scan(sgn, s_cmul, s_cmul, 1.0,
                                         OP.mult, OP.bypass)

            ph = [ppool.tile([128, 512], F32, name=f"ph{t}") for t in range(3)]
            pw = [ppool.tile([128, 512], F32, name=f"pw{t}") for t in range(3)]
            pdum = [ppool.tile([128, 256], F32, name=f"pd{t}") for t in range(2)]

            # warm the activation-function tables while inputs load
            warm = apool.tile([128, 1], BF16, name="warm")
            nc.vector.memset(warm, 1.0)
            nc.scalar.activation(warm, warm, AF.Sigmoid)
            nc.scalar.activation(warm, warm, AF.Sqrt)
            for e in range(P):
                d2 = wpool.tile([128, FD], BF16, tag="d2", name="d2")
                wsq = wpool.tile([128, FD], BF16, tag="wsq", name="wsq")
                for t, (o, ln) in enumerate(CHUNKS):
                    wh = s_wst[:, e * 256 : e * 256 + 128]
                    ww = s_wst[:, e * 256 + 128 : e * 256 + 256]
                    nc.tensor.matmul(ph[t][:, :ln], wh, s_mov[:, o : o + ln],
                                     start=True, stop=True)
                    nc.tensor.matmul(pw[t][:, :ln], ww, s_mov[:, o : o + ln],
                                     start=True, stop=True)
                for t, (o, ln) in enumerate(CHUNKS):
                    nc.scalar.activation(wsq[:, o : o + ln], pw[t][:, :ln],
                                         AF.Square)
                    nc.vector._custom_dve(
                        EDGE_D2, out=d2[:, o : o + ln],
                        in0=ph[t][:, :ln], in1=wsq[:, o : o + ln],
                        s0=s_ckh[:, e : e + 1])
                nc.vector.tensor_tensor(macc, macc, d2, OP.max)
                for di in range(2):
                    nc.tensor.matmul(pdum[di], s_wst[:, 0:128],
                                     s_mov[:, 0:256], start=True, stop=True)

            # mask = sigmoid(-100 * sgn * sqrt(-macc))  (macc holds -d2),
            # computed per free-dim half so the tree can start early
            sig = apool.tile([128, FD], BF16, name="sig")
            HFD = FD // 2
            prev = [None, None]
            for xh in range(2):
                sl = slice(xh * HFD, (xh + 1) * HFD)
                nc.scalar.activation(macc[:, sl], macc[:, sl], AF.Sqrt,
                                     scale=-1.0)
                nc.vector.tensor_tensor(macc[:, sl], macc[:, sl], sgn[:, sl],
                                        OP.mult)
                prev[xh] = nc.scalar.activation(sig[:, sl], macc[:, sl],
                                                AF.Sigmoid, scale=-SHARP)
            scur = S
            while scur > 1:
                k = scur // 2
                npar = k * NCH
                src0 = (scur - k) * NCH
                tmp = wpool.tile([npar, FD], BF16, tag="tree", name="tmp")
                for xh in range(2):
                    sl = slice(xh * HFD, (xh + 1) * HFD)
                    d_i = nc.sync.dma_start(out=tmp[:npar, sl],
                                            in_=sig[src0 : src0 + npar, sl])
                    if prev[xh] is not None:
                        add_dep_helper(d_i.ins, prev[xh].ins,
                                       reason="tree round reads prior max")
                    t_i = nc.vector.tensor_tensor(
                        sig[:npar, sl], sig[:npar, sl], tmp[:npar, sl], OP.max)
                    add_dep_helper(t_i.ins, d_i.ins, reason="tree max after dma")
                    prev[xh] = t_i
                scur -= k

            # comb2 = the 64 local grid rows, twice (partitions (dhalf, h)),
            # gathered by 4 parallel DMAs straight out of sig's slot 0
            comb2 = apool.tile([128, V], BF16, name="comb2")
            cdmas = []
            for base in (0, 64):
                d_a = nc.sync.dma_start(out=comb2[base : base + 55],
                                        in_=sig[0:5])
                d_b = nc.sync.dma_start(out=comb2[base + 55 : base + 64],
                                        in_=sig[5:6, 0 : 9 * V])
                cdmas += [d_a, d_b]
            for d_i in cdmas:
                for xh in range(2):
                    add_dep_helper(d_i.ins, prev[xh].ins,
                                   reason="comb reads final tree max")

            # extrusion: staged[p=(dhalf,h), (d16,w)] = comb2[p,w]*dvv[p,d]
            HD = 16
            for i in range(4):
                stg = wpool.tile([128, HD, V], BF16, tag="stg", name="stg")
                eng = nc.gpsimd if i == 0 else nc.vector
                m_i = eng.tensor_tensor(
                    stg,
                    comb2.unsqueeze(1).broadcast_to([128, HD, V]),
                    s_dvv[:, i * HD : (i + 1) * HD].unsqueeze(2).broadcast_to(
                        [128, HD, V]),
                    OP.mult)
                for d_i in cdmas:
                    add_dep_helper(m_i.ins, d_i.ins,
                                   reason="staging reads comb2")
                nc.sync.dma_start(out=vox[:, i * HD * V : (i + 1) * HD * V],
                                  in_=stg)

    nc.compile()
    return nc


_NC_CACHE = {}


def _get_nc(S):
    if S not in _NC_CACHE:
        _NC_CACHE[S] = _build_device(S)
    return _NC_CACHE[S]


# ----------------------------------------------------------------------------
# Host-side: polygon -> per-edge linear-form coefficients + crossing parity
# ----------------------------------------------------------------------------


def _poly_coeffs(poly):
    vmask = poly.sum(axis=1) != 0.0
    K = int(vmask.sum())
    order = np.argsort((~vmask).astype(np.int32), kind="stable")
    pv = poly[order].astype(np.float64)
    idx = np.arange(P)
    nxt = np.where(idx == K - 1, 0, idx + 1)
    v0 = pv
    v1 = pv[nxt]
    valid_e = idx < K if K >= 3 else np.zeros(P, bool)

    ex = v1[:, 0] - v0[:, 0]
    ey = v1[:, 1] - v0[:, 1]
    s2 = ex * ex + ey * ey + EPS
    k = np.sqrt(s2)

    hx = -ex / k
    hy = -ey / k
    hc = (v0[:, 0] * ex + v0[:, 1] * ey) / k + k / 2.0
    wx = -ey / k
    wy = ex / k
    wc = (ey * v0[:, 0] - ex * v0[:, 1]) / k

    hx = np.where(valid_e, hx, 0.0)
    hy = np.where(valid_e, hy, 0.0)
    hc = np.where(valid_e, hc, 1e3)
    wx = np.where(valid_e, wx, 0.0)
    wy = np.where(valid_e, wy, 0.0)
    wc = np.where(valid_e, wc, 0.0)
    khalf = np.where(valid_e, k / 2.0, 0.0)

    # crossing columns, f32 ops mirroring the reference bit-for-bit:
    # thr[e, y] = #{grid columns j with inter_x > x_j}; 0 when !y_crosses
    x32 = np.arange(V, dtype=np.float32) / np.float32(V - 1)
    y32 = x32
    x0 = v0[:, 0].astype(np.float32)[:, None]
    y0 = v0[:, 1].astype(np.float32)[:, None]
    x1 = v1[:, 0].astype(np.float32)[:, None]
    y1 = v1[:, 1].astype(np.float32)[:, None]
    yrow = y32[None, :]
    yc = ((y0 <= yrow) & (y1 > yrow)) | ((y1 <= yrow) & (y0 > yrow))
    t = (yrow - y0) / (y1 - y0 + np.float32(EPS))
    ix = x0 + (x1 - x0) * t                                   # (P, V) f32
    yc = yc & valid_e[:, None]
    thr = (ix[:, :, None] > x32[None, None, :]).sum(axis=2)   # (P, V) ints
    thr = np.where(yc, thr, 0)

    return dict(hx=hx, hy=hy, hc=hc, wx=wx, wy=wy, wc=wc, khalf=khalf,
                thr=thr)


def _parity_tables(thr):
    """Per-row crossing-parity histogram for one polygon.
    Returns (pm, rowpar): pm[y, j] = (-1)^{Htilde[y, j]} with
    Htilde[y, 0] = #{thr >= 1}, Htilde[y, j>=1] = #{thr == j}; the running
    product of row y's prefix has the parity of pixel (y, j)'s crossing
    count. rowpar[y] = parity of the whole row's Htilde sum."""
    Ht = np.zeros((V, V), np.int64)
    for y in range(V):
        th = thr[:, y]
        hist = np.bincount(th[(th >= 1) & (th <= V - 1)], minlength=V)
        Ht[y, 1:] = hist[1:]
        Ht[y, 0] = int((th >= 1).sum())
    pm = np.where(Ht % 2 == 1, -1.0, 1.0).astype(np.float32)
    rowpar = (Ht.sum(axis=1) % 2).astype(np.int64)
    return pm, rowpar


# ----------------------------------------------------------------------------
# Host entry point
# ----------------------------------------------------------------------------

LAST_RESULTS = None


def kernel(polygons, attributes, validity_scores, _trace=False):
    global LAST_RESULTS
    polygons = np.asarray(polygons)
    attributes = np.asarray(attributes)
    validity_scores = np.asarray(validity_scores)
    B, N, _, _ = polygons.shape
    assert (B, N) == (4, 32)

    valid_lists = [[n for n in range(N) if validity_scores[b, n] >= 0.5]
                   for b in range(B)]
    S = max(2, max(len(v) for v in valid_lists))
    assert S * NCH <= 128, f"too many valid polygons: {S}"
    nc = _get_nc(S)

    norm = np.clip(attributes[:, 0].astype(np.float32), 0.0, 1.0)
    hv = np.clip(np.rint(norm * np.float32(V)).astype(np.int32), 1, V)

    # per-(b, poly) precompute shared by both half-cores
    coeffs = {}
    parity = {}
    for b in range(B):
        for n in valid_lists[b]:
            cf = _poly_coeffs(np.asarray(polygons[b, n], np.float32))
            coeffs[(b, n)] = cf
            parity[(b, n)] = _parity_tables(cf["thr"])

    # moving tile: rows (x, j, 1) in free order f = j*V + c
    x32 = np.arange(V, dtype=np.float32) / np.float32(V - 1)
    movt = np.zeros((3, FD), np.float32)
    movt[0] = np.tile(x32, YY)
    movt[1] = np.repeat(np.arange(YY, dtype=np.float32), V)
    movt[2] = 1.0

    in_maps = []
    for c in range(NCORES):
        b, half = c // 2, c % 2
        plist = valid_lists[b]

        wstv = np.zeros((3, P * 2 * 128), np.float64)
        ckhv = np.zeros((128, P), np.float64)
        cmulv = np.ones((128, FD), np.float32)
        for p in range(128):
            s, ch = p // NCH, p % NCH
            if s < len(plist):
                cf = coeffs[(b, plist[s])]
                y0 = (half * HALF + ch * YY) / 127.0
                for e in range(P):
                    o = e * 256
                    wstv[0, o + p] = cf["hx"][e]
                    wstv[1, o + p] = cf["hy"][e] / 127.0
                    wstv[2, o + p] = cf["hy"][e] * y0 + cf["hc"][e]
                    wstv[0, o + 128 + p] = cf["wx"][e]
                    wstv[1, o + 128 + p] = cf["wy"][e] / 127.0
                    wstv[2, o + 128 + p] = cf["wy"][e] * y0 + cf["wc"][e]
                ckhv[p] = cf["khalf"]
                pm, rowpar = parity[(b, plist[s])]
                run = 0
                for j in range(YY):
                    row = half * HALF + ch * YY + j
                    if row >= V:
                        break
                    cmulv[p, j * V : (j + 1) * V] = pm[row]
                    if j > 0 and run % 2 == 1:
                        cmulv[p, j * V] = -cmulv[p, j * V]
                        run = 0
                    run += int(rowpar[row])
            else:
                for e in range(P):
                    wstv[2, e * 256 + p] = 1e3

        dvvv = np.zeros((128, HALF), np.float32)
        dmask = (np.arange(V) < hv[b]).astype(np.float32)
        for p in range(128):
            dh = p // 64
            dvvv[p] = dmask[dh * 64 : (dh + 1) * 64]

        import ml_dtypes
        in_maps.append({
            "mov": movt.astype(np.float32),
            "wst": wstv.astype(np.float32),
            "ckh": ckhv.astype(np.float32),
            "cmul": cmulv.astype(ml_dtypes.bfloat16),
            "dvv": dvvv.astype(ml_dtypes.bfloat16),
        })

    res = run_bass_kernel_spmd(nc, in_maps, core_ids=list(range(NCORES)),
                               trace=_trace)
    LAST_RESULTS = res

    out = np.zeros((B, V, V, V), np.float32)
    for c in range(NCORES):
        b, half = c // 2, c % 2
        a = np.asarray(res.results[c]["vox"]).astype(np.float32)
        a = a.reshape(2, 64, HALF, V)            # [dhalf, h, d', w]
        out[b, :, half * HALF : (half + 1) * HALF, :] = (
            a.transpose(0, 2, 1, 3).reshape(V, HALF, V))
    return np.ascontiguousarray(out)


# revision 18
# speedup vs baseline: 2.3359x; 1.2685x over previous
"""Trainium2 Bass kernel for DifferentiableExtrusion (v2 design).

voxels[b,d,h,w] = depth_mask[b,d] * max_n(valid_n * sigmoid(-100*sdf_n(h,w)))
B=4, N=32 polygons (P=16 vertices), V=128 grid, D=128.

Sharding: 8 cores = (b, row-half). Each core computes ALL valid polygons of
batch b over HALF the grid rows (64 rows), so no cross-core combine is
needed: each core locally max-reduces over its polygon slots and writes its
own [D, 64, W] block of the output (bf16; host converts to f32).

Per-core layout: 128 partitions = S poly slots x 6 row-chunks of YY=11 rows
(chunk bases ch*11 cover local rows 0..65; rows 64,65 are computed but
discarded). Free dim = 11*128 = 1408 pixels.

Per edge e (16 iterations):
  - PE (fp32r matmuls, K=3 against a [x; j; 1] moving tile): h and w linear
    forms into PSUM. Per-partition coefficients come from host-packed
    stationary tiles (the row base y0 is folded into the constant term).
    Filler matmuls keep the PE p-state ramped.
  - DVE: one custom fused op per FD chunk: d2 = relu(|h| - khalf)^2 + w^2
    (f32 PSUM in -> bf16 out). The custom op is registered at import time
    via the documented dve_ops extension point.
  - Pool (gpsimd): macc = min(macc, d2).

Inside test: host quantizes edge/row crossings exactly like the reference
(bit-for-bit f32 compares) and emits a per-pixel +-1 multiplier tile; one
Pool mult-scan turns it into the crossing-parity sign sgn (+1 outside).

Tail: r = sqrt(macc) [ACT], rs = r*sgn [DVE], sig = sigmoid(-100*rs) [ACT],
tree-max over poly slots (strided partition DMAs + DVE max, overlapping
free-dim halves), extrusion staged[p=(dhalf,h), (d',w)] = comb*depth [DVE],
partition-aligned bf16 output DMAs.
"""

import numpy as np

import concourse.bacc as bacc
import concourse.tile as tile
from concourse import mybir
from concourse import dve_ops
from concourse.dve_spec import (Spec, Src0, Src1, C0, Zero, Bin, maxx, sq,
                                lower, _has_src1, AluOp as DAlu)
from concourse.dve_uop import DveOpSpec
from concourse.bass_utils import run_bass_kernel_spmd
from concourse.tile_rust import add_dep_helper

V = 128
P = 16
HALF = 64          # grid rows per core
YY = 11            # rows per partition chunk
NCH = 6            # chunks per polygon (6*11 = 66 >= 64)
FD = YY * V        # 1408 free elements per partition
NPOS = NCH * YY    # 66 (j,ch) row positions (64 real rows + 2 junk)
SHARP = 100.0
EPS = 1e-8
NCORES = 8

F32 = mybir.dt.float32
F32R = mybir.dt.float32r
BF16 = mybir.dt.bfloat16
AF = mybir.ActivationFunctionType
OP = mybir.AluOpType

# FD chunking for PSUM banks (each chunk one 2KB bank; fp32r needs >= 256)
CHUNKS = [(0, 512), (512, 512), (1024, 384)]

# ----------------------------------------------------------------------------
# Custom DVE op: d2 = relu(|h| - c)^2 + w^2  in one instruction
# ----------------------------------------------------------------------------


def _register_d2_op():
    # d2 = relu(|h| - c)^2 + wsq, with h in PSUM and wsq (= w^2, squared on
    # the Activation engine) in SBUF — only one PSUM source is HW-legal.
    name = "EDGE_NEGD2_ANT"
    if name in dve_ops._SUB_OPCODE_FOR_NAME:
        for op in dve_ops.OPS:
            if op.name == name:
                return op
    spec = Spec(
        body=(Zero - sq(maxx(Bin(DAlu.ABSOLUTE_DIFF, Src0, Zero) - C0, Zero)))
        - Src1,
        reference=lambda in0, in1, s0, s1, imm2:
            -(np.maximum(np.abs(in0) - s0, 0.0, dtype=np.float32) ** 2) - in1,
    )
    row = max(dve_ops._SUB_OPCODE_FOR_NAME.values()) + 1
    assert row < 0x20, "no free custom-DVE opcode rows"
    shas = {}
    for ver in ("v3", "v4"):
        uops = lower(spec, ver=ver)
        shas[ver] = DveOpSpec(name=name, opcode=row, uops=uops,
                              rd1_en=_has_src1(spec)).sha(ver)
    op = dve_ops.DveOp(name, spec, subdim=False, uops_sha=shas)
    dve_ops._SUB_OPCODE_FOR_NAME[name] = row
    dve_ops.OPS.append(op)
    dve_ops.CUSTOM_DVE_SPECS[name] = spec
    return op


EDGE_D2 = _register_d2_op()

# ----------------------------------------------------------------------------
# Device module
# ----------------------------------------------------------------------------


def _build_device(S):
    nc = bacc.Bacc()
    mov = nc.declare_dram_parameter("mov", [3, FD], F32R, isOutput=False)
    wst = nc.declare_dram_parameter("wst", [3, P * 2 * 128], F32R, isOutput=False)
    ckh = nc.declare_dram_parameter("ckh", [128, P], F32, isOutput=False)
    cmul = nc.declare_dram_parameter("cmul", [128, FD], BF16, isOutput=False)
    dvv = nc.declare_dram_parameter("dvv", [128, V], BF16, isOutput=False)
    idn = nc.declare_dram_parameter("idn", [128, 128], BF16, isOutput=False)
    vox = nc.declare_dram_parameter("vox", [128, V * NPOS], BF16, isOutput=True)

    with tile.TileContext(nc) as tc:
        with (
            tc.tile_pool(name="const", bufs=1) as cpool,
            tc.tile_pool(name="work", bufs=2) as wpool,
            tc.tile_pool(name="acc", bufs=1) as apool,
            tc.tile_pool(name="ps", bufs=1, space="PSUM") as ppool,
        ):
            s_mov = cpool.tile([3, FD], F32R, name="s_mov")
            s_wst = cpool.tile([3, P * 2 * 128], F32R, name="s_wst")
            s_ckh = cpool.tile([128, P], F32, name="s_ckh")
            s_cmul = cpool.tile([128, FD], BF16, name="s_cmul")
            s_dvv = cpool.tile([128, V], BF16, name="s_dvv")
            s_idn = cpool.tile([128, 128], BF16, name="s_idn")
            nc.sync.dma_start(out=s_idn, in_=idn[:, :])
            nc.sync.dma_start(out=s_mov, in_=mov[:, :])
            nc.sync.dma_start(out=s_wst, in_=wst[:, :])
            nc.sync.dma_start(out=s_ckh, in_=ckh[:, :])
            nc.sync.dma_start(out=s_cmul, in_=cmul[:, :])
            nc.sync.dma_start(out=s_dvv, in_=dvv[:, :])

            macc = apool.tile([128, FD], BF16, name="macc")
            sgn = apool.tile([128, FD], BF16, name="sgn")
            nc.gpsimd.memset(macc, -1e9)
            # crossing-parity sign via one multiplicative scan
            nc.vector.tensor_tensor_scan(sgn, s_cmul, s_cmul, 1.0,
                                         OP.mult, OP.bypass)

            ph = [ppool.tile([128, 512], F32, name=f"ph{t}") for t in range(3)]
            pw = [ppool.tile([128, 512], F32, name=f"pw{t}") for t in range(3)]

            def pe_keepalive():
                # identity transpose into the shared pt rotation: keeps the
                # PE p-state from dropping between edge bursts
                pk = ppool.tile([128, 128], BF16, tag="pt", name="pk", bufs=2)
                nc.tensor.transpose(pk, s_idn, s_idn)

            # warm the activation-function tables while inputs load
            warm = apool.tile([128, 1], BF16, name="warm")
            nc.vector.memset(warm, 1.0)
            nc.scalar.activation(warm, warm, AF.Sigmoid)
            nc.scalar.activation(warm, warm, AF.Sqrt)
            for e in range(P):
                d2 = wpool.tile([128, FD], BF16, tag="d2", name="d2")
                wsq = wpool.tile([128, FD], BF16, tag="wsq", name="wsq")
                for t, (o, ln) in enumerate(CHUNKS):
                    wh = s_wst[:, e * 256 : e * 256 + 128]
                    ww = s_wst[:, e * 256 + 128 : e * 256 + 256]
                    nc.tensor.matmul(ph[t][:, :ln], wh, s_mov[:, o : o + ln],
                                     start=True, stop=True)
                    nc.tensor.matmul(pw[t][:, :ln], ww, s_mov[:, o : o + ln],
                                     start=True, stop=True)
                for t, (o, ln) in enumerate(CHUNKS):
                    nc.scalar.activation(wsq[:, o : o + ln], pw[t][:, :ln],
                                         AF.Square)
                    nc.vector._custom_dve(
                        EDGE_D2, out=d2[:, o : o + ln],
                        in0=ph[t][:, :ln], in1=wsq[:, o : o + ln],
                        s0=s_ckh[:, e : e + 1])
                nc.vector.tensor_tensor(macc, macc, d2, OP.max)
                for _ in range(3):
                    pe_keepalive()

            # mask = sigmoid(-100 * sgn * sqrt(-macc))  (macc holds -d2),
            # computed per free-dim half so the tree can start early
            sig = apool.tile([128, FD], BF16, name="sig")
            HFD = FD // 2
            prev = [None, None]
            for xh in range(2):
                sl = slice(xh * HFD, (xh + 1) * HFD)
                nc.scalar.activation(macc[:, sl], macc[:, sl], AF.Sqrt,
                                     scale=-1.0)
                nc.vector.tensor_tensor(macc[:, sl], macc[:, sl], sgn[:, sl],
                                        OP.mult)
                prev[xh] = nc.scalar.activation(sig[:, sl], macc[:, sl],
                                                AF.Sigmoid, scale=-SHARP)
            # combine over poly slots: PE-transpose each 128-column block
            # (partition <-> free), then one DVE max-reduce over the slot
            # axis via a strided AP. comb_T[w, j*6+ch] = combined mask of
            # grid row ch*11+j at column w. No DMAs involved.
            comb_T = apool.tile([128, NPOS], BF16, name="comb_T")
            red_insts = []
            for j in range(YY):
                pt = ppool.tile([128, 128], BF16, tag="pt", name="pt", bufs=2)
                t_i = nc.tensor.transpose(pt, sig[:, j * V : (j + 1) * V],
                                          s_idn)
                if j * V < HFD:
                    add_dep_helper(t_i.ins, prev[0].ins, reason="reads sig h0")
                if (j + 1) * V > HFD:
                    add_dep_helper(t_i.ins, prev[1].ins, reason="reads sig h1")
                r_i = nc.vector.tensor_reduce(
                    comb_T[:, j * NCH : (j + 1) * NCH],
                    pt[:, 0 : S * NCH].rearrange("p (s c) -> p c s", c=NCH),
                    mybir.AxisListType.X, OP.max)
                add_dep_helper(r_i.ins, t_i.ins, reason="reduce reads transpose")
                red_insts.append(r_i)

            # extrusion in transposed layout: staged[w, (d, pos)] =
            # comb_T[w, pos] * depth[d]; host maps pos = j*6+ch -> row.
            DCH = V // 4
            for i in range(4):
                stg = wpool.tile([128, DCH, NPOS], BF16, tag="stg", name="stg",
                                 bufs=4)
                eng = nc.gpsimd if i == 0 else nc.vector
                m_i = eng.tensor_tensor(
                    stg,
                    comb_T.unsqueeze(1).broadcast_to([128, DCH, NPOS]),
                    s_dvv[:, i * DCH : (i + 1) * DCH].unsqueeze(2).broadcast_to(
                        [128, DCH, NPOS]),
                    OP.mult)
                for r_i in red_insts:
                    add_dep_helper(m_i.ins, r_i.ins,
                                   reason="staging reads comb_T")
                nc.sync.dma_start(
                    out=vox[:, i * DCH * NPOS : (i + 1) * DCH * NPOS],
                    in_=stg)

    nc.compile()
    return nc


_NC_CACHE = {}


def _get_nc(S):
    if S not in _NC_CACHE:
        _NC_CACHE[S] = _build_device(S)
    return _NC_CACHE[S]


# ----------------------------------------------------------------------------
# Host-side: polygon -> per-edge linear-form coefficients + crossing parity
# ----------------------------------------------------------------------------


def _poly_coeffs(poly):
    vmask = poly.sum(axis=1) != 0.0
    K = int(vmask.sum())
    order = np.argsort((~vmask).astype(np.int32), kind="stable")
    pv = poly[order].astype(np.float64)
    idx = np.arange(P)
    nxt = np.where(idx == K - 1, 0, idx + 1)
    v0 = pv
    v1 = pv[nxt]
    valid_e = idx < K if K >= 3 else np.zeros(P, bool)

    ex = v1[:, 0] - v0[:, 0]
    ey = v1[:, 1] - v0[:, 1]
    s2 = ex * ex + ey * ey + EPS
    k = np.sqrt(s2)

    hx = -ex / k
    hy = -ey / k
    hc = (v0[:, 0] * ex + v0[:, 1] * ey) / k + k / 2.0
    wx = -ey / k
    wy = ex / k
    wc = (ey * v0[:, 0] - ex * v0[:, 1]) / k

    hx = np.where(valid_e, hx, 0.0)
    hy = np.where(valid_e, hy, 0.0)
    hc = np.where(valid_e, hc, 1e3)
    wx = np.where(valid_e, wx, 0.0)
    wy = np.where(valid_e, wy, 0.0)
    wc = np.where(valid_e, wc, 0.0)
    khalf = np.where(valid_e, k / 2.0, 0.0)

    # crossing columns, f32 ops mirroring the reference bit-for-bit:
    # thr[e, y] = #{grid columns j with inter_x > x_j}; 0 when !y_crosses
    x32 = np.arange(V, dtype=np.float32) / np.float32(V - 1)
    y32 = x32
    x0 = v0[:, 0].astype(np.float32)[:, None]
    y0 = v0[:, 1].astype(np.float32)[:, None]
    x1 = v1[:, 0].astype(np.float32)[:, None]
    y1 = v1[:, 1].astype(np.float32)[:, None]
    yrow = y32[None, :]
    yc = ((y0 <= yrow) & (y1 > yrow)) | ((y1 <= yrow) & (y0 > yrow))
    t = (yrow - y0) / (y1 - y0 + np.float32(EPS))
    ix = x0 + (x1 - x0) * t                                   # (P, V) f32
    yc = yc & valid_e[:, None]
    thr = (ix[:, :, None] > x32[None, None, :]).sum(axis=2)   # (P, V) ints
    thr = np.where(yc, thr, 0)

    return dict(hx=hx, hy=hy, hc=hc, wx=wx, wy=wy, wc=wc, khalf=khalf,
                thr=thr)


def _parity_tables(thr):
    """Per-row crossing-parity histogram for one polygon.
    Returns (pm, rowpar): pm[y, j] = (-1)^{Htilde[y, j]} with
    Htilde[y, 0] = #{thr >= 1}, Htilde[y, j>=1] = #{thr == j}; the running
    product of row y's prefix has the parity of pixel (y, j)'s crossing
    count. rowpar[y] = parity of the whole row's Htilde sum."""
    Ht = np.zeros((V, V), np.int64)
    for y in range(V):
        th = thr[:, y]
        hist = np.bincount(th[(th >= 1) & (th <= V - 1)], minlength=V)
        Ht[y, 1:] = hist[1:]
        Ht[y, 0] = int((th >= 1).sum())
    pm = np.where(Ht % 2 == 1, -1.0, 1.0).astype(np.float32)
    rowpar = (Ht.sum(axis=1) % 2).astype(np.int64)
    return pm, rowpar


# ----------------------------------------------------------------------------
# Host entry point
# ----------------------------------------------------------------------------

LAST_RESULTS = None


def kernel(polygons, attributes, validity_scores, _trace=False):
    global LAST_RESULTS
    polygons = np.asarray(polygons)
    attributes = np.asarray(attributes)
    validity_scores = np.asarray(validity_scores)
    B, N, _, _ = polygons.shape
    assert (B, N) == (4, 32)

    valid_lists = [[n for n in range(N) if validity_scores[b, n] >= 0.5]
                   for b in range(B)]
    S = max(2, max(len(v) for v in valid_lists))
    assert S * NCH <= 128, f"too many valid polygons: {S}"
    nc = _get_nc(S)

    norm = np.clip(attributes[:, 0].astype(np.float32), 0.0, 1.0)
    hv = np.clip(np.rint(norm * np.float32(V)).astype(np.int32), 1, V)

    # per-(b, poly) precompute shared by both half-cores
    coeffs = {}
    parity = {}
    for b in range(B):
        for n in valid_lists[b]:
            cf = _poly_coeffs(np.asarray(polygons[b, n], np.float32))
            coeffs[(b, n)] = cf
            parity[(b, n)] = _parity_tables(cf["thr"])

    # moving tile: rows (x, j, 1) in free order f = j*V + c
    x32 = np.arange(V, dtype=np.float32) / np.float32(V - 1)
    movt = np.zeros((3, FD), np.float32)
    movt[0] = np.tile(x32, YY)
    movt[1] = np.repeat(np.arange(YY, dtype=np.float32), V)
    movt[2] = 1.0

    in_maps = []
    for c in range(NCORES):
        b, half = c // 2, c % 2
        plist = valid_lists[b]

        wstv = np.zeros((3, P * 2 * 128), np.float64)
        ckhv = np.zeros((128, P), np.float64)
        cmulv = np.ones((128, FD), np.float32)
        for p in range(128):
            s, ch = p // NCH, p % NCH
            if s < len(plist):
                cf = coeffs[(b, plist[s])]
                y0 = (half * HALF + ch * YY) / 127.0
                for e in range(P):
                    o = e * 256
                    wstv[0, o + p] = cf["hx"][e]
                    wstv[1, o + p] = cf["hy"][e] / 127.0
                    wstv[2, o + p] = cf["hy"][e] * y0 + cf["hc"][e]
                    wstv[0, o + 128 + p] = cf["wx"][e]
                    wstv[1, o + 128 + p] = cf["wy"][e] / 127.0
                    wstv[2, o + 128 + p] = cf["wy"][e] * y0 + cf["wc"][e]
                ckhv[p] = cf["khalf"]
                pm, rowpar = parity[(b, plist[s])]
                run = 0
                for j in range(YY):
                    row = half * HALF + ch * YY + j
                    if row >= V:
                        break
                    cmulv[p, j * V : (j + 1) * V] = pm[row]
                    if j > 0 and run % 2 == 1:
                        cmulv[p, j * V] = -cmulv[p, j * V]
                        run = 0
                    run += int(rowpar[row])
            else:
                for e in range(P):
                    wstv[2, e * 256 + p] = 1e3

        dmask = (np.arange(V) < hv[b]).astype(np.float32)
        dvvv = np.tile(dmask, (128, 1))

        import ml_dtypes
        in_maps.append({
            "mov": movt.astype(np.float32),
            "wst": wstv.astype(np.float32),
            "ckh": ckhv.astype(np.float32),
            "cmul": cmulv.astype(ml_dtypes.bfloat16),
            "dvv": dvvv.astype(ml_dtypes.bfloat16),
            "idn": np.eye(128, dtype=np.float32).astype(ml_dtypes.bfloat16),
        })

    res = run_bass_kernel_spmd(nc, in_maps, core_ids=list(range(NCORES)),
                               trace=_trace)
    LAST_RESULTS = res

    out = np.zeros((B, V, V, V), np.float32)
    r_arange = np.arange(HALF)
    pos_for_r = (r_arange % YY) * NCH + r_arange // YY
    for c in range(NCORES):
        b, half = c // 2, c % 2
        a = np.asarray(res.results[c]["vox"]).astype(np.float32)
        a = a.reshape(V, V, NPOS)                # [w, d, pos]
        out[b, :, half * HALF : (half + 1) * HALF, :] = (
            a[:, :, pos_for_r].transpose(1, 2, 0))
    return np.ascontiguousarray(out)
